# revision 3
# baseline (speedup 1.0000x reference)
"""Trainium2 Bass kernel v2: classical single-head attention layer.

reference math:
    qkv = x @ w_qkv.T        # x [8192, 512], w_qkv [192, 512]
    q, k, v = split(qkv, 3)  # each [8192, 64]
    out = softmax(q @ k.T / 8) @ v   # [8192, 64]

Sharding: Q row-blocks across 8 cores (1024 rows each); K/V replicated.
Two NEFF passes:
  pass 1 (per core c, all bf16): project x[:, c-block]^T -> Q^T/K^T
          ([128,1024] bf16: rows 0:64 Q^T, 64:128 K^T) and V in the
          pass-2 V' SBUF image ([128, 8*VP_W] bf16, ones at col 64).
  host:   byte-level concat/fold of K^T and V' across cores (no casts).
  pass 2 (per core c): flash-style attention for the core's 1024 queries:
          S^T[key,q] chunks on PE (bf16 in, fp32 psum) with h0/h1 row-tile
          alternation, exp split across ACT (exact) / DVE / Pool
          (Schraudolph bf16) straight from PSUM, P^T@V' on PE with a
          ones-column producing the softmax denominator in row 64, then
          transpose + reciprocal-scale on DVE.
"""

import math
import os
from contextlib import ExitStack

import ml_dtypes
import numpy as np

import concourse.bass as bass
import concourse.mybir as mybir
import concourse.tile as tile
from concourse import bacc
from concourse.bass_utils import run_bass_kernel_spmd
from concourse.masks import make_identity

F32 = mybir.dt.float32
BF16 = mybir.dt.bfloat16
I16 = mybir.dt.int16

N = 8192          # sequence length
D_IN = 512        # input features
D = 64            # head dim
NC = 8            # cores
SEQ_C = N // NC   # 1024 queries per core
SCALE = 1.0 / math.sqrt(D)
BF = ml_dtypes.bfloat16

VP_W = 80         # V' chunk stride (65 used, 32B-aligned starts)

# Schraudolph bf16 exp: bf16_bits(exp(x)) ~= x*SCH_C1 + SCH_C2 (int16 round)
SCH_C1 = 128.0 / math.log(2.0)
SCH_C2 = 127.0 * 128.0 - 366393.0 / 65536.0

# pass-2 key-chunk processing order: alternate h0 (keys 0:4096) / h1
# (keys 4096:8192) chunks so consecutive S matmuls sit on different PE
# row halves, and group them to match the chunked kt/vp input DMAs.
CHUNK_ORDER = []
for _i in range(32):
    CHUNK_ORDER += [_i, 32 + _i]
CHUNK_POS = [0] * 64  # chunk id -> position in processing order
for _m, _j in enumerate(CHUNK_ORDER):
    CHUNK_POS[_j] = _m

GRP = 3                       # key chunks per exp batch (3 psum banks)
# exp engine split within each GRP*512 psum image: [ACT | DVE | POOL] cols
# aligned to 512-col chunk boundaries so each PV matmul depends on exactly
# one exp producer (ACT covers chunks 0-1 of the group, DVE chunk 2)
EXP_ACT = int(os.environ.get("EXP_ACT", "1024"))
EXP_DVE = int(os.environ.get("EXP_DVE", "512"))
# pool gets the rest
# PV pairing: chunk top/bot halves run as (64c,65o) tiles at row positions
# 0/64 concurrently, accumulating into SEPARATE psum banks (same-bank
# concurrent accumulation is rejected by the hardware), summed on DVE.
PV_PAIR = os.environ.get("PV_PAIR", "1") == "1"

LAST_RESULTS = []
_CACHE = {}


def _build_pass1():
    """Projection pass (bf16): xt [512,1024], wt [512,192] ->
    qk [128,1024] bf16, vp [128, 8*VP_W] bf16 (V' image with ones col)."""
    nc = bacc.Bacc("TRN2", target_bir_lowering=False, debug=False, num_devices=NC)
    xt_d = nc.dram_tensor("xt", [D_IN, SEQ_C], BF16, kind="ExternalInput")
    wt_d = nc.dram_tensor("wt", [D_IN, 3 * D], BF16, kind="ExternalInput")
    qk_d = nc.dram_tensor("qk", [128, SEQ_C], BF16, kind="ExternalOutput")
    vp_d = nc.dram_tensor("vp", [128, 8 * VP_W], BF16, kind="ExternalOutput")

    with tile.TileContext(nc) as tc, ExitStack() as ctx:
        sb = ctx.enter_context(tc.tile_pool(name="sb", bufs=1))
        ps_a = ctx.enter_context(tc.tile_pool(name="ps_a", bufs=2, space="PSUM"))
        ps_b = ctx.enter_context(tc.tile_pool(name="ps_b", bufs=4, space="PSUM"))

        # w^T as [128, 4 * 192]
        wt_sb = sb.tile([128, 4 * 3 * D], BF16)
        nc.sync.dma_start(
            wt_sb[:].rearrange("p (i o) -> p i o", i=4),
            wt_d.ap().rearrange("(i p) o -> p i o", p=128),
        )
        # x^T chunks as separate tiles, split across both DMA queues
        xt_sb = []
        for i in range(4):
            t = sb.tile([128, SEQ_C], BF16, tag=f"xt{i}")
            eng = nc.sync if i % 2 == 0 else nc.scalar
            eng.dma_start(t[:], xt_d[i * 128 : (i + 1) * 128, :])
            xt_sb.append(t)

        qk_sb = sb.tile([128, SEQ_C], BF16)
        vp_sb = sb.tile([128, 8 * VP_W], BF16)
        nc.vector.memset(vp_sb[:], 0.0)

        # Q^T/K^T: psum [128, 512] = sum_i WqkT_i.T @ xT_i
        for sblk in range(SEQ_C // 512):
            a = ps_a.tile([128, 512], F32)
            for i in range(4):
                nc.tensor.matmul(
                    a[:],
                    wt_sb[:, i * 192 : i * 192 + 128],
                    xt_sb[i][:, sblk * 512 : sblk * 512 + 512],
                    start=(i == 0),
                    stop=(i == 3),
                )
            nc.vector.tensor_copy(qk_sb[:, sblk * 512 : sblk * 512 + 512], a[:])
            nc.sync.dma_start(
                qk_d[:, sblk * 512 : sblk * 512 + 512],
                qk_sb[:, sblk * 512 : sblk * 512 + 512],
            )

        # V: psum [128 seq, 64] = sum_i xT_i(seq tile).T @ WvT_i
        for st in range(8):
            b = ps_b.tile([128, D], F32)
            for i in range(4):
                nc.tensor.matmul(
                    b[:],
                    xt_sb[i][:, st * 128 : st * 128 + 128],
                    wt_sb[:, i * 192 + 128 : i * 192 + 192],
                    start=(i == 0),
                    stop=(i == 3),
                )
            nc.vector.tensor_copy(vp_sb[:, st * VP_W : st * VP_W + D], b[:])
            nc.gpsimd.memset(vp_sb[:, st * VP_W + D : st * VP_W + D + 1], 1.0)

        nc.sync.dma_start(vp_d[:, :], vp_sb[:])

    nc.compile()
    return nc


def _build_pass2():
    """Attention pass per core.

    inputs : qt  [64, 1024] bf16 (Q^T; loaded into both partition halves)
             kt2 [128, 4096] bf16 (K^T folded: rows 0:64 keys 0:4096,
                 rows 64:128 keys 4096:8192)
             vp  [128, 64*VP_W] bf16 (V' image, processing order)
    output : out [1024, 64] f32
    """
    nc = bacc.Bacc("TRN2", target_bir_lowering=False, debug=False, num_devices=NC)
    qt_d = nc.dram_tensor("qt", [D, SEQ_C], BF16, kind="ExternalInput")
    kt_d = nc.dram_tensor("kt2", [128, N // 2], BF16, kind="ExternalInput")
    vp_d = nc.dram_tensor("vp", [128, (N // 128) * VP_W], BF16, kind="ExternalInput")
    out_d = nc.dram_tensor("out", [SEQ_C, D], F32, kind="ExternalOutput")

    n_chunks = N // 128

    with tile.TileContext(nc) as tc, ExitStack() as ctx:
        sb = ctx.enter_context(tc.tile_pool(name="sb", bufs=1))
        p_pool = ctx.enter_context(tc.tile_pool(name="pT", bufs=4))
        o_sb_pool = ctx.enter_context(tc.tile_pool(name="osb", bufs=2))
        fin_pool = ctx.enter_context(tc.tile_pool(name="fin", bufs=4))
        s_pool = ctx.enter_context(tc.tile_pool(name="sT", bufs=2, space="PSUM"))
        o_pool = ctx.enter_context(tc.tile_pool(name="oac", bufs=1, space="PSUM"))

        # inputs first: qt into both halves, kt2 in 8 pieces, vp in 4,
        # split across the two hwdge queues (sync + scalar)
        qt_sb = sb.tile([128, SEQ_C], BF16)
        nc.sync.dma_start(qt_sb[0:64, :], qt_d[:, :])
        nc.scalar.dma_start(qt_sb[64:128, :], qt_d[:, :])
        kt_sb = sb.tile([128, N // 2], BF16)
        vp_sb = sb.tile([128, (N // 128) * VP_W], BF16)
        KP = 8
        kw = (N // 2) // KP
        VPP = 4
        vw = ((N // 128) * VP_W) // VPP

        def kt_dma(eng, i):
            eng.dma_start(
                kt_sb[:, i * kw : (i + 1) * kw], kt_d[:, i * kw : (i + 1) * kw]
            )

        def vp_dma(eng, i):
            eng.dma_start(
                vp_sb[:, i * vw : (i + 1) * vw], vp_d[:, i * vw : (i + 1) * vw]
            )

        # issue order tuned for earliest first-group start:
        # sync: kt0, kt1, kt3, kt5, kt7, vp2 ; scalar: vp0, kt2, kt4, kt6, vp1, vp3
        kt_dma(nc.sync, 0)
        vp_dma(nc.scalar, 0)
        kt_dma(nc.sync, 1)
        kt_dma(nc.scalar, 2)
        kt_dma(nc.sync, 3)
        kt_dma(nc.scalar, 4)
        kt_dma(nc.sync, 5)
        kt_dma(nc.scalar, 6)
        kt_dma(nc.sync, 7)
        vp_dma(nc.scalar, 1)
        vp_dma(nc.sync, 2)
        vp_dma(nc.scalar, 3)

        ident = sb.tile([128, 128], F32)
        make_identity(nc, ident[:])
        # preload the exp table while input DMAs are in flight
        scratch = fin_pool.tile([1, 1], F32, tag="scr")
        nc.vector.memset(scratch[:], 0.0)
        nc.scalar.activation(
            scratch[:], scratch[:], mybir.ActivationFunctionType.Exp
        )

        def kt_slice(j):
            half = 64 * (j // 32)
            col = (j % 32) * 128
            return kt_sb[half : half + 64, col : col + 128]

        def vp_slice(j):
            off = CHUNK_POS[j] * VP_W
            return vp_sb[:, off : off + D + 1]

        exp_f = mybir.ActivationFunctionType.Exp
        n_grp = (n_chunks + GRP - 1) // GRP

        for qblk in range(SEQ_C // 512):
            if PV_PAIR:
                o_top = o_pool.tile([128, 512], F32, tag="otop")
                o_bot = o_pool.tile([128, 512], F32, tag="obot")
                o_ps = (o_top, o_bot)
            else:
                o_ps = o_pool.tile([128, 512], F32, tag="otop")
            q0 = qblk * 512

            # software pipeline: S(g) -> PV(g-1) on PE; exp(g) on ACT/DVE/POOL
            prev = None  # (s_ps, p_sb, gsz, g)
            for g in range(n_grp):
                g0 = g * GRP
                gsz = min(GRP, n_chunks - g0)
                s_ps = s_pool.tile([128, GRP * 512], F32, tag="sT")
                for u in range(gsz):
                    j = CHUNK_ORDER[g0 + u]
                    half = 64 * (j // 32)
                    nc.tensor.matmul(
                        s_ps[:, u * 512 : (u + 1) * 512],
                        kt_slice(j),
                        qt_sb[half : half + 64, q0 : q0 + 512],
                        start=True,
                        stop=True,
                    )
                # exp of this group (engine-split over the free range)
                p_sb = p_pool.tile([128, GRP * 512], BF16, tag="pT")
                w = gsz * 512
                a_end = min(EXP_ACT, w)
                d_end = min(a_end + EXP_DVE, w)
                if a_end > 0:
                    nc.scalar.activation(
                        p_sb[:, :a_end], s_ps[:, :a_end], exp_f, scale=SCALE
                    )
                if d_end > a_end:
                    nc.vector.tensor_scalar(
                        p_sb[:, a_end:d_end].bitcast(I16),
                        s_ps[:, a_end:d_end],
                        SCH_C1 * SCALE,
                        SCH_C2,
                        op0=mybir.AluOpType.mult,
                        op1=mybir.AluOpType.add,
                    )
                if w > d_end:
                    nc.gpsimd.tensor_scalar(
                        p_sb[:, d_end:w].bitcast(I16),
                        s_ps[:, d_end:w],
                        SCH_C1 * SCALE,
                        SCH_C2,
                        op0=mybir.AluOpType.mult,
                        op1=mybir.AluOpType.add,
                    )

                if prev is not None:
                    _emit_pv(nc, o_ps, vp_slice, prev, n_chunks)
                prev = (s_ps, p_sb, gsz, g0)
            _emit_pv(nc, o_ps, vp_slice, prev, n_chunks)

            # accumulators rows 0:64 = (P V)^T, row 64 = softmax denominator
            o_sb = o_sb_pool.tile([D + 1, 512], F32)
            if PV_PAIR:
                # tensor_tensor may read only one input from PSUM: stage o_bot
                # through SBUF on the (idle-ish) ACT engine first
                o_bsb = o_sb_pool.tile([D + 1, 512], F32, tag="obsb")
                nc.scalar.copy(o_bsb[:], o_bot[0 : D + 1, :])
                nc.vector.tensor_tensor(
                    o_sb[:],
                    o_top[0 : D + 1, :],
                    o_bsb[:],
                    mybir.AluOpType.add,
                )
                tp_bank = o_top
            else:
                # copy per 128-col piece so transpose t waits only on piece t
                for t in range(4):
                    nc.vector.tensor_copy(
                        o_sb[:, t * 128 : (t + 1) * 128],
                        o_ps[0 : D + 1, t * 128 : (t + 1) * 128],
                    )
                tp_bank = o_ps
            for t in range(4):
                tp = tp_bank[:, t * 128 : t * 128 + D + 1]
                nc.tensor.transpose(
                    tp,
                    o_sb[:, t * 128 : (t + 1) * 128],
                    ident[: D + 1, : D + 1],
                )
                rec = fin_pool.tile([128, 1], F32, tag="rec")
                nc.vector.reciprocal(rec[:], tp[:, D : D + 1])
                ot = fin_pool.tile([128, D], F32, tag="ot")
                nc.vector.tensor_scalar(
                    ot[:], tp[:, :D], rec[:], None, op0=mybir.AluOpType.mult
                )
                r0 = q0 + t * 128
                nc.sync.dma_start(out_d[r0 : r0 + 128, :], ot[:])

    nc.compile()
    return nc


def _emit_pv(nc, o_ps, vp_slice, prev, n_chunks):
    s_ps, p_sb, gsz, g0 = prev
    for u in range(gsz):
        j = CHUNK_ORDER[g0 + u]
        m = g0 + u
        first = m == 0
        last = m == n_chunks - 1
        vsl = vp_slice(j)
        psl = p_sb[:, u * 512 : (u + 1) * 512]
        if not PV_PAIR:
            nc.tensor.matmul(
                o_ps[0 : D + 1, :],
                vsl,
                psl,
                start=first,
                stop=last,
                skip_group_check=True,
            )
        else:
            o_top, o_bot = o_ps
            nc.tensor.matmul(
                o_top[0 : D + 1, :],
                vsl[0:64, :],
                psl[0:64, :],
                start=first,
                stop=last,
                skip_group_check=True,
            )
            nc.tensor.matmul(
                o_bot[0 : D + 1, :],
                vsl[64:128, :],
                psl[64:128, :],
                start=first,
                stop=last,
                skip_group_check=True,
            )


def kernel(x: np.ndarray, w_qkv: np.ndarray) -> np.ndarray:
    global LAST_RESULTS
    LAST_RESULTS = []
    x = np.asarray(x, dtype=np.float32)
    w_qkv = np.asarray(w_qkv, dtype=np.float32)

    if "p1" not in _CACHE:
        _CACHE["p1"] = _build_pass1()
    if "p2" not in _CACHE:
        _CACHE["p2"] = _build_pass2()

    xt = np.ascontiguousarray(x.T.astype(BF))     # [512, 8192] bf16
    wt = np.ascontiguousarray(w_qkv.T.astype(BF))  # [512, 192] bf16

    in_maps1 = [
        {
            "xt": np.ascontiguousarray(xt[:, c * SEQ_C : (c + 1) * SEQ_C]),
            "wt": wt,
        }
        for c in range(NC)
    ]
    res1 = run_bass_kernel_spmd(_CACHE["p1"], in_maps1, core_ids=list(range(NC)))
    LAST_RESULTS.append(res1)

    qk = [res1.results[c]["qk"] for c in range(NC)]   # [128, 1024] bf16
    kt_full = np.concatenate([m[64:128] for m in qk], axis=1)  # [64, 8192]
    kt2 = np.ascontiguousarray(
        np.concatenate([kt_full[:, : N // 2], kt_full[:, N // 2 :]], axis=0)
    )

    # vp: core c's chunk i (global chunk j=c*8+i) placed at processing
    # position CHUNK_POS[j]
    vp = np.empty((128, (N // 128) * VP_W), dtype=BF)
    for c in range(NC):
        vpc = res1.results[c]["vp"]  # [128, 8*VP_W]
        for i in range(8):
            m = CHUNK_POS[c * 8 + i]
            vp[:, m * VP_W : (m + 1) * VP_W] = vpc[:, i * VP_W : (i + 1) * VP_W]

    in_maps2 = [
        {
            "qt": np.ascontiguousarray(qk[c][0:64]),
            "kt2": kt2,
            "vp": vp,
        }
        for c in range(NC)
    ]
    res2 = run_bass_kernel_spmd(_CACHE["p2"], in_maps2, core_ids=list(range(NC)))
    LAST_RESULTS.append(res2)

    out = np.concatenate([res2.results[c]["out"] for c in range(NC)], axis=0)
    return out.astype(np.float32)


# revision 4
# speedup vs baseline: 1.0269x; 1.0269x over previous
"""Trainium2 Bass kernel: classical single-head attention layer.

reference math:
    qkv = x @ w_qkv.T        # x [8192, 512], w_qkv [192, 512]
    q, k, v = split(qkv, 3)  # each [8192, 64]
    out = softmax(q @ k.T / 8) @ v   # [8192, 64]

Sharding: Q row-blocks across 8 cores (1024 rows each); K/V replicated.
Two NEFF passes:
  pass 1 (per core c, all bf16): project x[:, c-block]^T -> Q^T/K^T
          ([128,1024] bf16: rows 0:64 Q^T, 64:128 K^T) and V in the
          pass-2 V' SBUF image ([128, 8*VP_W] bf16, ones at col 64).
  host:   byte-level concat/fold of K^T and V' across cores (no casts).
  pass 2 (per core c): flash-style attention for the core's 1024 queries:
          S^T[key,q] chunks on PE (bf16 in, fp32 psum) ordered so
          consecutive matmuls alternate PE row-tile halves (h0/h1 run
          concurrently when unthrottled); exp reads PSUM directly, split
          chunk-aligned between ACT (exact exp, 2 chunks/group) and DVE
          (Schraudolph bf16 via one tensor_scalar, 1 chunk/group);
          P^T@V' on PE as paired 64-contract half-chunk matmuls into two
          separate PSUM accumulators (same-bank concurrent accumulation
          is illegal), with a ones-column in V' producing the softmax
          denominator in row 64; accumulators summed via ACT+DVE, then
          transpose + reciprocal-scale + store.
"""

import math
import os
from contextlib import ExitStack

import ml_dtypes
import numpy as np

import concourse.bass as bass
import concourse.mybir as mybir
import concourse.tile as tile
from concourse import bacc
from concourse.bass_utils import run_bass_kernel_spmd
from concourse.masks import make_identity

F32 = mybir.dt.float32
BF16 = mybir.dt.bfloat16
I16 = mybir.dt.int16

N = 8192          # sequence length
D_IN = 512        # input features
D = 64            # head dim
NC = 8            # cores
SEQ_C = N // NC   # 1024 queries per core
SCALE = 1.0 / math.sqrt(D)
BF = ml_dtypes.bfloat16

VP_W = 80         # V' chunk stride (65 used, 32B-aligned starts)

# Schraudolph bf16 exp: bf16_bits(exp(x)) ~= x*SCH_C1 + SCH_C2 (int16 round)
SCH_C1 = 128.0 / math.log(2.0)
SCH_C2 = 127.0 * 128.0 - 366393.0 / 65536.0

# pass-2 key-chunk processing order: alternate h0 (keys 0:4096) / h1
# (keys 4096:8192) chunks so consecutive S matmuls sit on different PE
# row halves, and group them to match the chunked kt/vp input DMAs.
CHUNK_ORDER = []
for _i in range(32):
    CHUNK_ORDER += [_i, 32 + _i]
CHUNK_POS = [0] * 64  # chunk id -> position in processing order
for _m, _j in enumerate(CHUNK_ORDER):
    CHUNK_POS[_j] = _m

GRP = 3                       # key chunks per exp batch (3 psum banks)
# exp engine split within each GRP*512 psum image: [ACT | DVE | POOL] cols
# aligned to 512-col chunk boundaries so each PV matmul depends on exactly
# one exp producer (ACT covers chunks 0-1 of the group, DVE chunk 2)
EXP_ACT = int(os.environ.get("EXP_ACT", "1024"))
EXP_DVE = int(os.environ.get("EXP_DVE", "512"))
# pool gets the rest
# PV pairing: chunk top/bot halves run as (64c,65o) tiles at row positions
# 0/64 concurrently, accumulating into SEPARATE psum banks (same-bank
# concurrent accumulation is rejected by the hardware), summed on DVE.
PV_PAIR = os.environ.get("PV_PAIR", "1") == "1"

LAST_RESULTS = []
_CACHE = {}


def _build_pass1():
    """Projection pass (bf16): xt [512,1024], wt [512,192] ->
    qk [128,1024] bf16, vp [128, 8*VP_W] bf16 (V' image with ones col)."""
    nc = bacc.Bacc("TRN2", target_bir_lowering=False, debug=False, num_devices=NC)
    xt_d = nc.dram_tensor("xt", [D_IN, SEQ_C], BF16, kind="ExternalInput")
    wt_d = nc.dram_tensor("wt", [D_IN, 3 * D], BF16, kind="ExternalInput")
    qk_d = nc.dram_tensor("qk", [128, SEQ_C], BF16, kind="ExternalOutput")
    vp_d = nc.dram_tensor("vp", [128, 8 * VP_W], BF16, kind="ExternalOutput")

    with tile.TileContext(nc) as tc, ExitStack() as ctx:
        sb = ctx.enter_context(tc.tile_pool(name="sb", bufs=1))
        ps_a = ctx.enter_context(tc.tile_pool(name="ps_a", bufs=2, space="PSUM"))
        ps_b = ctx.enter_context(tc.tile_pool(name="ps_b", bufs=4, space="PSUM"))

        # w^T as [128, 4 * 192]
        wt_sb = sb.tile([128, 4 * 3 * D], BF16)
        nc.sync.dma_start(
            wt_sb[:].rearrange("p (i o) -> p i o", i=4),
            wt_d.ap().rearrange("(i p) o -> p i o", p=128),
        )
        # x^T chunks as separate tiles, split across both DMA queues
        xt_sb = []
        for i in range(4):
            t = sb.tile([128, SEQ_C], BF16, tag=f"xt{i}")
            eng = nc.sync if i % 2 == 0 else nc.scalar
            eng.dma_start(t[:], xt_d[i * 128 : (i + 1) * 128, :])
            xt_sb.append(t)

        qk_sb = sb.tile([128, SEQ_C], BF16)
        vp_sb = sb.tile([128, 8 * VP_W], BF16)
        nc.vector.memset(vp_sb[:], 0.0)

        # Q^T/K^T: psum [128, 512] = sum_i WqkT_i.T @ xT_i
        for sblk in range(SEQ_C // 512):
            a = ps_a.tile([128, 512], F32)
            for i in range(4):
                nc.tensor.matmul(
                    a[:],
                    wt_sb[:, i * 192 : i * 192 + 128],
                    xt_sb[i][:, sblk * 512 : sblk * 512 + 512],
                    start=(i == 0),
                    stop=(i == 3),
                )
            nc.vector.tensor_copy(qk_sb[:, sblk * 512 : sblk * 512 + 512], a[:])
            nc.sync.dma_start(
                qk_d[:, sblk * 512 : sblk * 512 + 512],
                qk_sb[:, sblk * 512 : sblk * 512 + 512],
            )

        # V: psum [128 seq, 64] = sum_i xT_i(seq tile).T @ WvT_i
        for st in range(8):
            b = ps_b.tile([128, D], F32)
            for i in range(4):
                nc.tensor.matmul(
                    b[:],
                    xt_sb[i][:, st * 128 : st * 128 + 128],
                    wt_sb[:, i * 192 + 128 : i * 192 + 192],
                    start=(i == 0),
                    stop=(i == 3),
                )
            nc.vector.tensor_copy(vp_sb[:, st * VP_W : st * VP_W + D], b[:])
            nc.gpsimd.memset(vp_sb[:, st * VP_W + D : st * VP_W + D + 1], 1.0)

        nc.sync.dma_start(vp_d[:, :], vp_sb[:])

    nc.compile()
    return nc


def _build_pass2():
    """Attention pass per core.

    inputs : qt  [64, 1024] bf16 (Q^T; loaded into both partition halves)
             kt2 [128, 4096] bf16 (K^T folded: rows 0:64 keys 0:4096,
                 rows 64:128 keys 4096:8192)
             vp  [128, 64*VP_W] bf16 (V' image, processing order)
    output : out [1024, 64] f32
    """
    nc = bacc.Bacc("TRN2", target_bir_lowering=False, debug=False, num_devices=NC)
    qt_d = nc.dram_tensor("qt", [D, SEQ_C], BF16, kind="ExternalInput")
    kt_d = nc.dram_tensor("kt2", [128, N // 2], BF16, kind="ExternalInput")
    vp_d = nc.dram_tensor("vp", [128, (N // 128) * VP_W], BF16, kind="ExternalInput")
    out_d = nc.dram_tensor("out", [SEQ_C, D], F32, kind="ExternalOutput")

    n_chunks = N // 128

    with tile.TileContext(nc) as tc, ExitStack() as ctx:
        sb = ctx.enter_context(tc.tile_pool(name="sb", bufs=1))
        p_pool = ctx.enter_context(tc.tile_pool(name="pT", bufs=4))
        o_sb_pool = ctx.enter_context(tc.tile_pool(name="osb", bufs=2))
        fin_pool = ctx.enter_context(tc.tile_pool(name="fin", bufs=4))
        s_pool = ctx.enter_context(tc.tile_pool(name="sT", bufs=2, space="PSUM"))
        o_pool = ctx.enter_context(tc.tile_pool(name="oac", bufs=1, space="PSUM"))

        # inputs first: qt into both halves, kt2 in 8 pieces, vp in 4,
        # split across the two hwdge queues (sync + scalar)
        qt_sb = sb.tile([128, SEQ_C], BF16)
        nc.sync.dma_start(qt_sb[0:64, :], qt_d[:, :])
        nc.scalar.dma_start(qt_sb[64:128, :], qt_d[:, :])
        kt_sb = sb.tile([128, N // 2], BF16)
        vp_sb = sb.tile([128, (N // 128) * VP_W], BF16)
        KP = 8
        kw = (N // 2) // KP
        VPP = 4
        vw = ((N // 128) * VP_W) // VPP

        def kt_dma(eng, i):
            eng.dma_start(
                kt_sb[:, i * kw : (i + 1) * kw], kt_d[:, i * kw : (i + 1) * kw]
            )

        def vp_dma(eng, i):
            eng.dma_start(
                vp_sb[:, i * vw : (i + 1) * vw], vp_d[:, i * vw : (i + 1) * vw]
            )

        # issue order tuned for earliest first-group start:
        # sync: kt0, kt1, kt3, kt5, kt7, vp2 ; scalar: vp0, kt2, kt4, kt6, vp1, vp3
        kt_dma(nc.sync, 0)
        vp_dma(nc.scalar, 0)
        kt_dma(nc.sync, 1)
        kt_dma(nc.scalar, 2)
        kt_dma(nc.sync, 3)
        kt_dma(nc.scalar, 4)
        kt_dma(nc.sync, 5)
        kt_dma(nc.scalar, 6)
        kt_dma(nc.sync, 7)
        vp_dma(nc.scalar, 1)
        vp_dma(nc.sync, 2)
        vp_dma(nc.scalar, 3)

        ident = sb.tile([128, 128], F32)
        make_identity(nc, ident[:])
        # preload the exp table while input DMAs are in flight
        scratch = fin_pool.tile([1, 1], F32, tag="scr")
        nc.vector.memset(scratch[:], 0.0)
        nc.scalar.activation(
            scratch[:], scratch[:], mybir.ActivationFunctionType.Exp
        )

        def kt_slice(j):
            half = 64 * (j // 32)
            col = (j % 32) * 128
            return kt_sb[half : half + 64, col : col + 128]

        def vp_slice(j):
            off = CHUNK_POS[j] * VP_W
            return vp_sb[:, off : off + D + 1]

        exp_f = mybir.ActivationFunctionType.Exp
        n_grp = (n_chunks + GRP - 1) // GRP

        for qblk in range(SEQ_C // 512):
            if PV_PAIR:
                o_top = o_pool.tile([128, 512], F32, tag="otop")
                o_bot = o_pool.tile([128, 512], F32, tag="obot")
                o_ps = (o_top, o_bot)
            else:
                o_ps = o_pool.tile([128, 512], F32, tag="otop")
            q0 = qblk * 512

            # software pipeline: S(g) -> PV(g-1) on PE; exp(g) on ACT/DVE/POOL
            prev = None  # (s_ps, p_sb, gsz, g)
            for g in range(n_grp):
                g0 = g * GRP
                gsz = min(GRP, n_chunks - g0)
                s_ps = s_pool.tile([128, GRP * 512], F32, tag="sT")
                for u in range(gsz):
                    j = CHUNK_ORDER[g0 + u]
                    half = 64 * (j // 32)
                    nc.tensor.matmul(
                        s_ps[:, u * 512 : (u + 1) * 512],
                        kt_slice(j),
                        qt_sb[half : half + 64, q0 : q0 + 512],
                        start=True,
                        stop=True,
                    )
                # exp of this group (engine-split over the free range)
                p_sb = p_pool.tile([128, GRP * 512], BF16, tag="pT")
                w = gsz * 512
                a_end = min(EXP_ACT, w)
                d_end = min(a_end + EXP_DVE, w)
                if a_end > 0:
                    nc.scalar.activation(
                        p_sb[:, :a_end], s_ps[:, :a_end], exp_f, scale=SCALE
                    )
                if d_end > a_end:
                    nc.vector.tensor_scalar(
                        p_sb[:, a_end:d_end].bitcast(I16),
                        s_ps[:, a_end:d_end],
                        SCH_C1 * SCALE,
                        SCH_C2,
                        op0=mybir.AluOpType.mult,
                        op1=mybir.AluOpType.add,
                    )
                if w > d_end:
                    nc.gpsimd.tensor_scalar(
                        p_sb[:, d_end:w].bitcast(I16),
                        s_ps[:, d_end:w],
                        SCH_C1 * SCALE,
                        SCH_C2,
                        op0=mybir.AluOpType.mult,
                        op1=mybir.AluOpType.add,
                    )

                if prev is not None:
                    _emit_pv(nc, o_ps, vp_slice, prev, n_chunks)
                prev = (s_ps, p_sb, gsz, g0)
            _emit_pv(nc, o_ps, vp_slice, prev, n_chunks)

            # accumulators rows 0:64 = (P V)^T, row 64 = softmax denominator
            o_sb = o_sb_pool.tile([D + 1, 512], F32)
            if PV_PAIR:
                # tensor_tensor may read only one input from PSUM: stage o_bot
                # through SBUF on the (idle-ish) ACT engine first
                o_bsb = o_sb_pool.tile([D + 1, 512], F32, tag="obsb")
                nc.scalar.copy(o_bsb[:], o_bot[0 : D + 1, :])
                nc.vector.tensor_tensor(
                    o_sb[:],
                    o_top[0 : D + 1, :],
                    o_bsb[:],
                    mybir.AluOpType.add,
                )
                tp_bank = o_top
            else:
                # copy per 128-col piece so transpose t waits only on piece t
                for t in range(4):
                    nc.vector.tensor_copy(
                        o_sb[:, t * 128 : (t + 1) * 128],
                        o_ps[0 : D + 1, t * 128 : (t + 1) * 128],
                    )
                tp_bank = o_ps
            for t in range(4):
                tp = tp_bank[:, t * 128 : t * 128 + D + 1]
                nc.tensor.transpose(
                    tp,
                    o_sb[:, t * 128 : (t + 1) * 128],
                    ident[: D + 1, : D + 1],
                )
                rec = fin_pool.tile([128, 1], F32, tag="rec")
                nc.vector.reciprocal(rec[:], tp[:, D : D + 1])
                ot = fin_pool.tile([128, D], F32, tag="ot")
                nc.vector.tensor_scalar(
                    ot[:], tp[:, :D], rec[:], None, op0=mybir.AluOpType.mult
                )
                r0 = q0 + t * 128
                nc.sync.dma_start(out_d[r0 : r0 + 128, :], ot[:])

    nc.compile()
    return nc


def _emit_pv(nc, o_ps, vp_slice, prev, n_chunks):
    s_ps, p_sb, gsz, g0 = prev
    for u in range(gsz):
        j = CHUNK_ORDER[g0 + u]
        m = g0 + u
        first = m == 0
        last = m == n_chunks - 1
        vsl = vp_slice(j)
        psl = p_sb[:, u * 512 : (u + 1) * 512]
        if not PV_PAIR:
            nc.tensor.matmul(
                o_ps[0 : D + 1, :],
                vsl,
                psl,
                start=first,
                stop=last,
                skip_group_check=True,
            )
        else:
            o_top, o_bot = o_ps
            nc.tensor.matmul(
                o_top[0 : D + 1, :],
                vsl[0:64, :],
                psl[0:64, :],
                start=first,
                stop=last,
                skip_group_check=True,
            )
            nc.tensor.matmul(
                o_bot[0 : D + 1, :],
                vsl[64:128, :],
                psl[64:128, :],
                start=first,
                stop=last,
                skip_group_check=True,
            )


def kernel(x: np.ndarray, w_qkv: np.ndarray) -> np.ndarray:
    global LAST_RESULTS
    LAST_RESULTS = []
    x = np.asarray(x, dtype=np.float32)
    w_qkv = np.asarray(w_qkv, dtype=np.float32)

    if "p1" not in _CACHE:
        _CACHE["p1"] = _build_pass1()
    if "p2" not in _CACHE:
        _CACHE["p2"] = _build_pass2()

    xt = np.ascontiguousarray(x.T.astype(BF))     # [512, 8192] bf16
    wt = np.ascontiguousarray(w_qkv.T.astype(BF))  # [512, 192] bf16

    in_maps1 = [
        {
            "xt": np.ascontiguousarray(xt[:, c * SEQ_C : (c + 1) * SEQ_C]),
            "wt": wt,
        }
        for c in range(NC)
    ]
    res1 = run_bass_kernel_spmd(_CACHE["p1"], in_maps1, core_ids=list(range(NC)))
    LAST_RESULTS.append(res1)

    qk = [res1.results[c]["qk"] for c in range(NC)]   # [128, 1024] bf16
    kt_full = np.concatenate([m[64:128] for m in qk], axis=1)  # [64, 8192]
    kt2 = np.ascontiguousarray(
        np.concatenate([kt_full[:, : N // 2], kt_full[:, N // 2 :]], axis=0)
    )

    # vp: core c's chunk i (global chunk j=c*8+i) placed at processing
    # position CHUNK_POS[j]
    vp = np.empty((128, (N // 128) * VP_W), dtype=BF)
    for c in range(NC):
        vpc = res1.results[c]["vp"]  # [128, 8*VP_W]
        for i in range(8):
            m = CHUNK_POS[c * 8 + i]
            vp[:, m * VP_W : (m + 1) * VP_W] = vpc[:, i * VP_W : (i + 1) * VP_W]

    in_maps2 = [
        {
            "qt": np.ascontiguousarray(qk[c][0:64]),
            "kt2": kt2,
            "vp": vp,
        }
        for c in range(NC)
    ]
    res2 = run_bass_kernel_spmd(_CACHE["p2"], in_maps2, core_ids=list(range(NC)))
    LAST_RESULTS.append(res2)

    out = np.concatenate([res2.results[c]["out"] for c in range(NC)], axis=0)
    return out.astype(np.float32)


# revision 5
# speedup vs baseline: 1.0524x; 1.0248x over previous
"""Trainium2 Bass kernel: classical single-head attention layer.

reference math:
    qkv = x @ w_qkv.T        # x [8192, 512], w_qkv [192, 512]
    q, k, v = split(qkv, 3)  # each [8192, 64]
    out = softmax(q @ k.T / 8) @ v   # [8192, 64]

Sharding: Q row-blocks across 8 cores (1024 rows each); K/V replicated.
Two NEFF passes:
  pass 1 (per core c, all bf16): project x[:, c-block]^T -> Q^T/K^T
          ([128,1024] bf16: rows 0:64 Q^T, 64:128 K^T) and V in the
          pass-2 V' SBUF image ([128, 8*VP_W] bf16, ones at col 64).
  host:   byte-level concat/fold of K^T and V' across cores (no casts).
  pass 2 (per core c): flash-style attention for the core's 1024 queries:
          S^T[key,q] chunks on PE (bf16 in, fp32 psum) ordered so
          consecutive matmuls alternate PE row-tile halves (h0/h1 run
          concurrently when unthrottled); exp reads PSUM directly, split
          chunk-aligned between ACT (exact exp, 2 chunks/group) and DVE
          (Schraudolph bf16 via one tensor_scalar, 1 chunk/group);
          P^T@V' on PE as paired 64-contract half-chunk matmuls into two
          separate PSUM accumulators (same-bank concurrent accumulation
          is illegal), with a ones-column in V' producing the softmax
          denominator in row 64; accumulators summed via ACT+DVE, then
          transpose + reciprocal-scale + store.
"""

import math
import os
from contextlib import ExitStack

import ml_dtypes
import numpy as np

import concourse.bass as bass
import concourse.mybir as mybir
import concourse.tile as tile
from concourse import bacc
from concourse.bass_utils import run_bass_kernel_spmd
from concourse.masks import make_identity

F32 = mybir.dt.float32
BF16 = mybir.dt.bfloat16
I16 = mybir.dt.int16

N = 8192          # sequence length
D_IN = 512        # input features
D = 64            # head dim
NC = 8            # cores
SEQ_C = N // NC   # 1024 queries per core
SCALE = 1.0 / math.sqrt(D)
BF = ml_dtypes.bfloat16

VP_W = 80         # V' chunk stride (65 used, 32B-aligned starts)

# Schraudolph bf16 exp: bf16_bits(exp(x)) ~= x*SCH_C1 + SCH_C2 (int16 round)
SCH_C1 = 128.0 / math.log(2.0)
SCH_C2 = 127.0 * 128.0 - 366393.0 / 65536.0

# pass-2 key-chunk processing order: alternate h0 (keys 0:4096) / h1
# (keys 4096:8192) chunks so consecutive S matmuls sit on different PE
# row halves, and group them to match the chunked kt/vp input DMAs.
CHUNK_ORDER = []
for _i in range(32):
    CHUNK_ORDER += [_i, 32 + _i]
CHUNK_POS = [0] * 64  # chunk id -> position in processing order
for _m, _j in enumerate(CHUNK_ORDER):
    CHUNK_POS[_j] = _m

GRP = 3                       # key chunks per exp batch (3 psum banks)
# exp engine split within each GRP*512 psum image: [ACT | DVE | POOL] cols
# aligned to 512-col chunk boundaries so each PV matmul depends on exactly
# one exp producer (ACT covers chunks 0-1 of the group, DVE chunk 2)
EXP_ACT = int(os.environ.get("EXP_ACT", "1024"))
EXP_DVE = int(os.environ.get("EXP_DVE", "512"))
# pool gets the rest
# PV pairing: chunk top/bot halves run as (64c,65o) tiles at row positions
# 0/64 concurrently, accumulating into SEPARATE psum banks (same-bank
# concurrent accumulation is rejected by the hardware), summed on DVE.
PV_PAIR = os.environ.get("PV_PAIR", "1") == "1"

LAST_RESULTS = []
_CACHE = {}


def _build_pass1():
    """Projection pass (bf16): xt [512,1024], wt [512,192] ->
    qk [128,1024] bf16, vp [128, 8*VP_W] bf16 (V' image with ones col)."""
    nc = bacc.Bacc("TRN2", target_bir_lowering=False, debug=False, num_devices=NC)
    xt_d = nc.dram_tensor("xt", [D_IN, SEQ_C], BF16, kind="ExternalInput")
    wt_d = nc.dram_tensor("wt", [128, 4 * 3 * D], BF16, kind="ExternalInput")
    qk_d = nc.dram_tensor("qk", [128, SEQ_C], BF16, kind="ExternalOutput")
    vp_d = nc.dram_tensor("vp", [128, 8 * VP_W], BF16, kind="ExternalOutput")

    with tile.TileContext(nc) as tc, ExitStack() as ctx:
        sb = ctx.enter_context(tc.tile_pool(name="sb", bufs=1))
        ps_a = ctx.enter_context(tc.tile_pool(name="ps_a", bufs=2, space="PSUM"))
        ps_b = ctx.enter_context(tc.tile_pool(name="ps_b", bufs=4, space="PSUM"))

        # wt is host-prearranged into the [128, 4*192] SBUF image: one
        # contiguous descriptor per partition (the strided rearrange DMA
        # completed only at ~12us, on the pass-1 critical path)
        wt_sb = sb.tile([128, 4 * 3 * D], BF16)
        nc.sync.dma_start(wt_sb[:], wt_d[:, :])
        # x^T chunks as separate tiles, split across both DMA queues
        xt_sb = []
        for i in range(4):
            t = sb.tile([128, SEQ_C], BF16, tag=f"xt{i}")
            eng = nc.sync if i % 2 == 0 else nc.scalar
            eng.dma_start(t[:], xt_d[i * 128 : (i + 1) * 128, :])
            xt_sb.append(t)

        qk_sb = sb.tile([128, SEQ_C], BF16)
        vp_sb = sb.tile([128, 8 * VP_W], BF16)
        nc.vector.memset(vp_sb[:], 0.0)

        # Q^T/K^T: psum [128, 512] = sum_i WqkT_i.T @ xT_i
        for sblk in range(SEQ_C // 512):
            a = ps_a.tile([128, 512], F32)
            for i in range(4):
                nc.tensor.matmul(
                    a[:],
                    wt_sb[:, i * 192 : i * 192 + 128],
                    xt_sb[i][:, sblk * 512 : sblk * 512 + 512],
                    start=(i == 0),
                    stop=(i == 3),
                )
            nc.vector.tensor_copy(qk_sb[:, sblk * 512 : sblk * 512 + 512], a[:])
            nc.sync.dma_start(
                qk_d[:, sblk * 512 : sblk * 512 + 512],
                qk_sb[:, sblk * 512 : sblk * 512 + 512],
            )

        # V: psum [128 seq, 64] = sum_i xT_i(seq tile).T @ WvT_i
        for st in range(8):
            b = ps_b.tile([128, D], F32)
            for i in range(4):
                nc.tensor.matmul(
                    b[:],
                    xt_sb[i][:, st * 128 : st * 128 + 128],
                    wt_sb[:, i * 192 + 128 : i * 192 + 192],
                    start=(i == 0),
                    stop=(i == 3),
                )
            nc.vector.tensor_copy(vp_sb[:, st * VP_W : st * VP_W + D], b[:])
            nc.gpsimd.memset(vp_sb[:, st * VP_W + D : st * VP_W + D + 1], 1.0)

        nc.sync.dma_start(vp_d[:, :], vp_sb[:])

    nc.compile()
    return nc


def _build_pass2():
    """Attention pass per core.

    inputs : qt  [64, 1024] bf16 (Q^T; loaded into both partition halves)
             kt2 [128, 4096] bf16 (K^T folded: rows 0:64 keys 0:4096,
                 rows 64:128 keys 4096:8192)
             vp  [128, 64*VP_W] bf16 (V' image, processing order)
    output : out [1024, 64] f32
    """
    nc = bacc.Bacc("TRN2", target_bir_lowering=False, debug=False, num_devices=NC)
    qt_d = nc.dram_tensor("qt", [D, SEQ_C], BF16, kind="ExternalInput")
    kt_d = nc.dram_tensor("kt2", [128, N // 2], BF16, kind="ExternalInput")
    vp_d = nc.dram_tensor("vp", [128, (N // 128) * VP_W], BF16, kind="ExternalInput")
    out_d = nc.dram_tensor("out", [SEQ_C, D], F32, kind="ExternalOutput")

    n_chunks = N // 128

    with tile.TileContext(nc) as tc, ExitStack() as ctx:
        sb = ctx.enter_context(tc.tile_pool(name="sb", bufs=1))
        p_pool = ctx.enter_context(tc.tile_pool(name="pT", bufs=4))
        o_sb_pool = ctx.enter_context(tc.tile_pool(name="osb", bufs=2))
        fin_pool = ctx.enter_context(tc.tile_pool(name="fin", bufs=4))
        s_pool = ctx.enter_context(tc.tile_pool(name="sT", bufs=2, space="PSUM"))
        o_pool = ctx.enter_context(tc.tile_pool(name="oac", bufs=1, space="PSUM"))

        # inputs first: qt into both halves, kt2 in 8 pieces, vp in 4,
        # split across the two hwdge queues (sync + scalar)
        qt_sb = sb.tile([128, SEQ_C], BF16)
        nc.sync.dma_start(qt_sb[0:64, :], qt_d[:, :])
        nc.scalar.dma_start(qt_sb[64:128, :], qt_d[:, :])
        kt_sb = sb.tile([128, N // 2], BF16)
        vp_sb = sb.tile([128, (N // 128) * VP_W], BF16)
        KP = 8
        kw = (N // 2) // KP
        VPP = 4
        vw = ((N // 128) * VP_W) // VPP

        def kt_dma(eng, i):
            eng.dma_start(
                kt_sb[:, i * kw : (i + 1) * kw], kt_d[:, i * kw : (i + 1) * kw]
            )

        def vp_dma(eng, i):
            eng.dma_start(
                vp_sb[:, i * vw : (i + 1) * vw], vp_d[:, i * vw : (i + 1) * vw]
            )

        # issue order tuned for earliest first-group start:
        # sync: kt0, kt1, kt3, kt5, kt7, vp2 ; scalar: vp0, kt2, kt4, kt6, vp1, vp3
        kt_dma(nc.sync, 0)
        vp_dma(nc.scalar, 0)
        kt_dma(nc.sync, 1)
        kt_dma(nc.scalar, 2)
        kt_dma(nc.sync, 3)
        kt_dma(nc.scalar, 4)
        kt_dma(nc.sync, 5)
        kt_dma(nc.scalar, 6)
        kt_dma(nc.sync, 7)
        vp_dma(nc.scalar, 1)
        vp_dma(nc.sync, 2)
        vp_dma(nc.scalar, 3)

        ident = sb.tile([128, 128], F32)
        make_identity(nc, ident[:])
        # preload the exp table while input DMAs are in flight
        scratch = fin_pool.tile([1, 1], F32, tag="scr")
        nc.vector.memset(scratch[:], 0.0)
        nc.scalar.activation(
            scratch[:], scratch[:], mybir.ActivationFunctionType.Exp
        )

        def kt_slice(j):
            half = 64 * (j // 32)
            col = (j % 32) * 128
            return kt_sb[half : half + 64, col : col + 128]

        def vp_slice(j):
            off = CHUNK_POS[j] * VP_W
            return vp_sb[:, off : off + D + 1]

        exp_f = mybir.ActivationFunctionType.Exp
        n_grp = (n_chunks + GRP - 1) // GRP

        for qblk in range(SEQ_C // 512):
            if PV_PAIR:
                o_top = o_pool.tile([128, 512], F32, tag="otop")
                o_bot = o_pool.tile([128, 512], F32, tag="obot")
                o_ps = (o_top, o_bot)
            else:
                o_ps = o_pool.tile([128, 512], F32, tag="otop")
            q0 = qblk * 512

            # software pipeline: S(g) -> PV(g-1) on PE; exp(g) on ACT/DVE/POOL
            prev = None  # (s_ps, p_sb, gsz, g)
            for g in range(n_grp):
                g0 = g * GRP
                gsz = min(GRP, n_chunks - g0)
                s_ps = s_pool.tile([128, GRP * 512], F32, tag="sT")
                for u in range(gsz):
                    j = CHUNK_ORDER[g0 + u]
                    half = 64 * (j // 32)
                    nc.tensor.matmul(
                        s_ps[:, u * 512 : (u + 1) * 512],
                        kt_slice(j),
                        qt_sb[half : half + 64, q0 : q0 + 512],
                        start=True,
                        stop=True,
                    )
                # exp of this group (engine-split over the free range)
                p_sb = p_pool.tile([128, GRP * 512], BF16, tag="pT")
                w = gsz * 512
                a_end = min(EXP_ACT, w)
                d_end = min(a_end + EXP_DVE, w)
                if a_end > 0:
                    nc.scalar.activation(
                        p_sb[:, :a_end], s_ps[:, :a_end], exp_f, scale=SCALE
                    )
                if d_end > a_end:
                    nc.vector.tensor_scalar(
                        p_sb[:, a_end:d_end].bitcast(I16),
                        s_ps[:, a_end:d_end],
                        SCH_C1 * SCALE,
                        SCH_C2,
                        op0=mybir.AluOpType.mult,
                        op1=mybir.AluOpType.add,
                    )
                if w > d_end:
                    nc.gpsimd.tensor_scalar(
                        p_sb[:, d_end:w].bitcast(I16),
                        s_ps[:, d_end:w],
                        SCH_C1 * SCALE,
                        SCH_C2,
                        op0=mybir.AluOpType.mult,
                        op1=mybir.AluOpType.add,
                    )

                if prev is not None:
                    _emit_pv(nc, o_ps, vp_slice, prev, n_chunks)
                prev = (s_ps, p_sb, gsz, g0)
            _emit_pv(nc, o_ps, vp_slice, prev, n_chunks)

            # accumulators rows 0:64 = (P V)^T, row 64 = softmax denominator
            o_sb = o_sb_pool.tile([D + 1, 512], F32)
            if PV_PAIR:
                # tensor_tensor may read only one input from PSUM: stage o_bot
                # through SBUF on the (idle-ish) ACT engine first
                o_bsb = o_sb_pool.tile([D + 1, 512], F32, tag="obsb")
                nc.scalar.copy(o_bsb[:], o_bot[0 : D + 1, :])
                nc.vector.tensor_tensor(
                    o_sb[:],
                    o_top[0 : D + 1, :],
                    o_bsb[:],
                    mybir.AluOpType.add,
                )
                tp_bank = o_top
            else:
                # copy per 128-col piece so transpose t waits only on piece t
                for t in range(4):
                    nc.vector.tensor_copy(
                        o_sb[:, t * 128 : (t + 1) * 128],
                        o_ps[0 : D + 1, t * 128 : (t + 1) * 128],
                    )
                tp_bank = o_ps
            for t in range(4):
                tp = tp_bank[:, t * 128 : t * 128 + D + 1]
                nc.tensor.transpose(
                    tp,
                    o_sb[:, t * 128 : (t + 1) * 128],
                    ident[: D + 1, : D + 1],
                )
                rec = fin_pool.tile([128, 1], F32, tag="rec")
                nc.vector.reciprocal(rec[:], tp[:, D : D + 1])
                ot = fin_pool.tile([128, D], F32, tag="ot")
                nc.vector.tensor_scalar(
                    ot[:], tp[:, :D], rec[:], None, op0=mybir.AluOpType.mult
                )
                r0 = q0 + t * 128
                nc.sync.dma_start(out_d[r0 : r0 + 128, :], ot[:])

    nc.compile()
    return nc


def _emit_pv(nc, o_ps, vp_slice, prev, n_chunks):
    s_ps, p_sb, gsz, g0 = prev
    for u in range(gsz):
        j = CHUNK_ORDER[g0 + u]
        m = g0 + u
        first = m == 0
        last = m == n_chunks - 1
        vsl = vp_slice(j)
        psl = p_sb[:, u * 512 : (u + 1) * 512]
        if not PV_PAIR:
            nc.tensor.matmul(
                o_ps[0 : D + 1, :],
                vsl,
                psl,
                start=first,
                stop=last,
                skip_group_check=True,
            )
        else:
            o_top, o_bot = o_ps
            nc.tensor.matmul(
                o_top[0 : D + 1, :],
                vsl[0:64, :],
                psl[0:64, :],
                start=first,
                stop=last,
                skip_group_check=True,
            )
            nc.tensor.matmul(
                o_bot[0 : D + 1, :],
                vsl[64:128, :],
                psl[64:128, :],
                start=first,
                stop=last,
                skip_group_check=True,
            )


def kernel(x: np.ndarray, w_qkv: np.ndarray) -> np.ndarray:
    global LAST_RESULTS
    LAST_RESULTS = []
    x = np.asarray(x, dtype=np.float32)
    w_qkv = np.asarray(w_qkv, dtype=np.float32)

    if "p1" not in _CACHE:
        _CACHE["p1"] = _build_pass1()
    if "p2" not in _CACHE:
        _CACHE["p2"] = _build_pass2()

    xt = np.ascontiguousarray(x.T.astype(BF))     # [512, 8192] bf16
    # wt in the [128, 4*192] SBUF image: wt[p, i*192+o] = w_qkv.T[i*128+p, o]
    wt = np.ascontiguousarray(
        w_qkv.T.astype(BF)
        .reshape(4, 128, 3 * D)
        .transpose(1, 0, 2)
        .reshape(128, 4 * 3 * D)
    )

    in_maps1 = [
        {
            "xt": np.ascontiguousarray(xt[:, c * SEQ_C : (c + 1) * SEQ_C]),
            "wt": wt,
        }
        for c in range(NC)
    ]
    res1 = run_bass_kernel_spmd(_CACHE["p1"], in_maps1, core_ids=list(range(NC)))
    LAST_RESULTS.append(res1)

    qk = [res1.results[c]["qk"] for c in range(NC)]   # [128, 1024] bf16
    kt_full = np.concatenate([m[64:128] for m in qk], axis=1)  # [64, 8192]
    kt2 = np.ascontiguousarray(
        np.concatenate([kt_full[:, : N // 2], kt_full[:, N // 2 :]], axis=0)
    )

    # vp: core c's chunk i (global chunk j=c*8+i) placed at processing
    # position CHUNK_POS[j]
    vp = np.empty((128, (N // 128) * VP_W), dtype=BF)
    for c in range(NC):
        vpc = res1.results[c]["vp"]  # [128, 8*VP_W]
        for i in range(8):
            m = CHUNK_POS[c * 8 + i]
            vp[:, m * VP_W : (m + 1) * VP_W] = vpc[:, i * VP_W : (i + 1) * VP_W]

    in_maps2 = [
        {
            "qt": np.ascontiguousarray(qk[c][0:64]),
            "kt2": kt2,
            "vp": vp,
        }
        for c in range(NC)
    ]
    res2 = run_bass_kernel_spmd(_CACHE["p2"], in_maps2, core_ids=list(range(NC)))
    LAST_RESULTS.append(res2)

    out = np.concatenate([res2.results[c]["out"] for c in range(NC)], axis=0)
    return out.astype(np.float32)
